# revision 63
# baseline (speedup 1.0000x reference)
"""Fused transformer block (LN -> QKV+RoPE -> attention -> out_proj) on 8
Trainium2 NeuronCores.

Sharding: batch (2-way) x heads (4-way) = 8 cores. Core c handles batch
b = c // 4 and the 4 heads starting at 4*(c%4). Each core produces the
out_proj partial sum over its 256 dh-dims; the host sums 4 partials per
batch and adds b_out.

Device math, per core (all matmuls bf16 in / fp32 PSUM accumulate; the
2e-2 tolerance absorbs it):
- x passed transposed: xT [D, S] bf16 (d on partitions, s free).
- all bulk loads issued up front across the sync/gpsimd/scalar DMA
  rings so no weight DMA queues behind a stats-dependent bounce DMA.
- LN stats via TensorE ones-matmuls with x^2 on ScalarE: both engines
  are idle in the pre-attention ramp, so stats pipeline behind the xT
  DMAs for free. mu kept negated: mean-centering is a K=1 matmul
  accumulation folded into the QKV PSUM group.
- q/k RoPE in [e, s] layout; one PSUM read per (e, sh): q is pre-scaled
  by rstd there (RoPE commutes with per-position scalars), k plain-cast;
  k's rstd is folded into exp's per-partition scale. The sin table's
  sign layout lets the rotate-half swap (partition-sliced SBUF->SBUF
  DMAs on the two HWDGE rings) run AFTER the sin multiply.
- software-pipelined emission (per-engine execution is program-order):
  qk0-sh0 + qk2 first (qk0's second s-half defers into the A(1,0)
  fill), then single-HEAD attention blocks (6 PSUM banks:
  pscore ring 2x2 + po 2), leaving 2 banks for the fill rings. V
  streams as fill steps inside the first attention block (v(t) lands
  just before AV(j=t)); qk(1)/qk(3) matmul+RoPE chunks and out_proj
  (i-half 0) tiles drip-feed into later j-loops so the PE works while
  ScalarE exps. ScalarE runs ONLY exp in steady state.
- scores^T[j,i] per (head, j-tile) with K=64; exp on ScalarE
  (scale = rstd_k[j]/8); o^T accumulated over j with lhsT = [v | 1]
  (M=65, row 64 = softmax denominators for free). po + denominator row
  are copied RAW to SBUF (DVE) right after stop; reciprocal (on the
  packed [1,2,SH] rows), DRAM-bounce broadcast, and in-place oT
  normalize run at the earliest (ih, et) completion, off the po-bank
  critical path; the very last normalization splits per head-half so
  half its DRAM round-trip hides inside the final attention block.
- out_proj: i-half 0 interleaved into the last attention blocks; i-half
  1 as a tail on a 3-deep PSUM ring with PSUM->SBUF copies alternating
  DVE/ScalarE (ScalarE is idle after the last exp); host reduces + adds
  b_out.

KREPS env (measurement only): repeats the whole body N times in one
NEFF so device time = d(marginal)/d(KREPS), cancelling the unstable
axon dispatch term. Default 1.
"""
import sys
sys.path.insert(0, "/opt/trn_rl_repo")
import numpy as np
import ml_dtypes
BF16 = ml_dtypes.bfloat16

B, S, D = 2, 2048, 1024
HEADS, HDIM = 16, 64
HALF = HDIM // 2
ROPE_THETA = 10000.0
N_CORES = 8
HPC = HEADS // 4            # heads per core = 4
EC = HPC * HDIM             # per-core q (or k, or v) width = 256
P = 128
NK = D // P                 # 8 d-tiles
NS = S // P                 # 16 s-tiles
VW = HDIM + 1               # v block width incl. ones column = 65
SH = S // 2                 # i-half width = 1024

_cache = {}


def _build():
    import os
    import contextlib
    import concourse.bass as bass
    import concourse.bacc as bacc
    import concourse.tile as tile
    from concourse import mybir
    fp32 = mybir.dt.float32
    bf16 = mybir.dt.bfloat16
    OP = mybir.AluOpType
    AF = mybir.ActivationFunctionType
    _abl = os.environ.get("ABLATE", "")

    nc = bacc.Bacc("TRN2", target_bir_lowering=False, debug=False,
                   enable_asserts=True, num_devices=N_CORES)

    xT = nc.dram_tensor("xT", [D, S], bf16, kind="ExternalInput").ap()
    wqkT = nc.dram_tensor("wqkT", [D, 2 * EC], bf16, kind="ExternalInput").ap()
    wvT = nc.dram_tensor("wvT", [D, EC], bf16, kind="ExternalInput").ap()
    woT = nc.dram_tensor("woT", [EC, D], bf16, kind="ExternalInput").ap()
    wsum_qk = nc.dram_tensor("wsum_qk", [2 * EC], bf16,
                             kind="ExternalInput").ap()
    wvsum = nc.dram_tensor("wvsum", [EC], fp32, kind="ExternalInput").ap()
    cosf = nc.dram_tensor("cosf", [P, S], fp32, kind="ExternalInput").ap()
    sinsg = nc.dram_tensor("sinsg", [P, S], fp32, kind="ExternalInput").ap()
    out = nc.dram_tensor("out", [S, D], bf16, kind="ExternalOutput").ap()

    wqk_r = wqkT.rearrange("(k p) e -> p k e", p=P)
    wv_r = wvT.rearrange("(k p) e -> p k e", p=P)
    wo_r = woT.rearrange("(k p) e -> p k e", p=P)

    reps = int(os.environ.get("KREPS", "1"))
    with tile.TileContext(nc) as tc:
     for _rep in range(reps):
      with tc.tile_pool(name=f"singles{_rep}", bufs=1) as singles, \
           tc.tile_pool(name=f"dram_scr{_rep}", bufs=1,
                        space="DRAM") as dram_scr:
        qk_sb = singles.tile([P, 4, S], bf16)             # 16KB/part
        v_sb = singles.tile([P, NS, HPC * VW], bf16)      # 8.1KB/part
        oT_sb = singles.tile([P, 2, S], bf16)             # raw (unnormalized)
        wo_sb = singles.tile([P, 2, D], bf16)
        den_sb = singles.tile([1, 2, 2, 2, SH], fp32)     # [ih, et, hp, i]
        rstdT = singles.tile([P, NS], fp32)
        muT = singles.tile([P, NS], bf16)
        muTf = singles.tile([P, NS], fp32)
        rstdT8 = singles.tile([P, NS], fp32)
        onep = singles.tile([P, 2], fp32)
        nc.vector.memset(onep[:], 1.0)
        nc.vector.memset(onep[0:1, 1:2], 1e-5)
        eps_b = singles.tile([P, 1], fp32)
        nc.vector.memset(eps_b[:], 1e-5)
        ones_rt = singles.tile([P, 1], bf16)
        nc.vector.tensor_copy(out=ones_rt[:], in_=onep[:, 0:1])
        ones_sb = ones_rt[:]
        eps_sb = onep[0:1, 1:2]
        # ones column of [v | 1]
        nc.vector.tensor_copy(
            out=v_sb[:].rearrange("p t (h w) -> p t h w", w=VW)[:, :, :,
                                                               HDIM:VW],
            in_=onep[:, 0:1].broadcast_to([P, NS, HPC, 1]))

        with tc.tile_pool(name="ph1a", bufs=1) as ph1a:
            # ---------------- bulk loads, consumption order ----------------
            xT_sb = ph1a.tile([P, NK, S], bf16)           # 32KB/part
            xT_r = xT.rearrange("(k p) s -> p k s", p=P)
            for k in range(NK):     # split across the two HWDGE rings
                eng = (nc.sync, nc.scalar)[k % 2]
                eng.dma_start(out=xT_sb[:, k, :], in_=xT_r[:, k, :])
            wv_sb = ph1a.tile([P, NK, EC], bf16)          # 4KB/part
            for k2 in range(2):
                nc.sync.dma_start(out=wv_sb[:, 4 * k2:4 * (k2 + 1), :],
                                  in_=wv_r[:, 4 * k2:4 * (k2 + 1), :])
            wqk_sb = ph1a.tile([P, 4, NK, P], bf16)       # 8KB/part
            for e in range(4):
                nc.scalar.dma_start(out=wqk_sb[:, e, :, :],
                                    in_=wqk_r[:, :, e * P:(e + 1) * P])
            wsqk_row = ph1a.tile([1, 2 * EC], bf16)
            nc.sync.dma_start(out=wsqk_row[:], in_=wsum_qk.unsqueeze(0))
            wsv_b = ph1a.tile([P, EC], fp32)
            nc.sync.dma_start(
                out=wsv_b[:],
                in_=bass.AP(tensor=wvsum.tensor, offset=wvsum.offset,
                            ap=[[0, P], [1, EC]]))
            cos_sb = ph1a.tile([P, S], fp32)
            sin_sb = ph1a.tile([P, S], fp32)
            nc.scalar.dma_start(out=cos_sb[:], in_=cosf)
            nc.sync.dma_start(out=sin_sb[:], in_=sinsg)
            for k in range(2):
                nc.sync.dma_start(out=wo_sb[:, k, :], in_=wo_r[:, k, :])
            rstd_b = ph1a.tile([P, S], fp32)
            mu_sb = ph1a.tile([1, S], bf16)           # holds -mu

            # ---------------- phase 0: LN stats ----------------
            with tc.tile_pool(name="p0ps_a", bufs=2, space="PSUM") as p0ps_a, \
                 tc.tile_pool(name="p0ps_b", bufs=1, space="PSUM") as p0ps_b, \
                 tc.tile_pool(name="p0scr", bufs=1) as p0scr, \
                 tc.tile_pool(name="p0tmp", bufs=2) as p0tmp:
                ssq_sb = p0scr.tile([1, S], fp32)
                rstd_sb = p0scr.tile([1, S], fp32)
                # stats PE matmuls + ScalarE Square both live in T0, where
                # both engines are otherwise idle (first exp is ~50us away);
                # they pipeline behind the xT DMAs for free.
                for c in range(4):
                    ps_sum = p0ps_a.tile([1, 512], fp32, tag="ps")
                    for k in range(NK):
                        nc.tensor.matmul(ps_sum[:], ones_sb,
                                         xT_sb[:, k, c * 512:(c + 1) * 512],
                                         start=(k == 0), stop=(k == NK - 1))
                    nc.vector.tensor_scalar_mul(mu_sb[:, c * 512:(c + 1) * 512],
                                                ps_sum[:], -1.0 / D)
                # bounce mu + gather its transpose NOW: the DMA round
                # trip overlaps the whole x^2/psq phase below.
                mu_d = dram_scr.tile([1, S], bf16)
                nc.sync.dma_start(out=mu_d[:], in_=mu_sb[:])
                _md = mu_d[:]
                nc.sync.dma_start(
                    out=muT[:],
                    in_=bass.AP(tensor=_md.tensor, offset=_md.offset,
                                ap=[[1, P], [P, NS]]))
                nc.vector.tensor_copy(out=muTf[:], in_=muT[:])
                psq = [p0ps_b.tile([1, 512], fp32, tag=f"psq{c}", name=f"psq{c}")
                       for c in range(4)]
                for k in range(NK):
                    for h2 in range(2):
                        xsq = p0tmp.tile([P, S // 2], bf16, tag="xsq")
                        nc.scalar.activation(
                            xsq[:], xT_sb[:, k, h2 * 1024:(h2 + 1) * 1024],
                            AF.Square)
                        for c in range(2):
                            ci = h2 * 2 + c
                            nc.tensor.matmul(psq[ci][:], ones_sb,
                                             xsq[:, c * 512:(c + 1) * 512],
                                             start=(k == 0), stop=(k == NK - 1),
                                             skip_group_check=True)
                for c in range(4):
                    nc.vector.tensor_copy(out=ssq_sb[:, c * 512:(c + 1) * 512],
                                          in_=psq[c][:])
                # rstd computed in the TRANSPOSED [128, NS] layout: the
                # row-form chain costs ~2us per [1, S] op (partition-
                # starved); here every op is free-size NS=16 (~70ns).
                # Bounce mu/ssq rows -> DRAM, gather transposed, compute
                # rstdT, bounce back, and gather the rstd_b broadcast
                # straight from the transposed DRAM image.
                ssq_d = dram_scr.tile([1, S], fp32)
                nc.sync.dma_start(out=ssq_d[:], in_=ssq_sb[:])
                _sd = ssq_d[:]
                ssqT = p0scr.tile([P, NS], fp32)
                nc.sync.dma_start(
                    out=ssqT[:],
                    in_=bass.AP(tensor=_sd.tensor, offset=_sd.offset,
                                ap=[[1, P], [P, NS]]))
                # var = ssq/D - mu^2 ; rstd = 1/sqrt(var + eps)
                nc.vector.tensor_mul(rstdT[:], muTf[:], muTf[:])
                nc.vector.scalar_tensor_tensor(out=rstdT[:], in0=ssqT[:],
                                               scalar=1.0 / D, in1=rstdT[:],
                                               op0=OP.mult, op1=OP.subtract)
                nc.scalar.activation(rstdT[:], rstdT[:], AF.Sqrt,
                                     bias=eps_b[:])
                nc.vector.reciprocal(out=rstdT[:], in_=rstdT[:])
                nc.vector.tensor_scalar_mul(rstdT8[:], rstdT[:],
                                            float(HDIM) ** -0.5)
                # broadcast form: bounce rstdT's [p, t] image and gather
                # rows with s = t*128 + p -> addr p*NS + t (3D AP).
                rstd_td = dram_scr.tile([1, S], fp32)
                _rt = rstd_td[:]
                # transposed WRITE: rstdT[p, t] lands at s = t*128 + p, so
                # DRAM holds rstd in natural s-order for the broadcasts.
                nc.sync.dma_start(
                    out=bass.AP(tensor=_rt.tensor, offset=_rt.offset,
                                ap=[[1, P], [P, NS]]),
                    in_=rstdT[:])
                for hh in range(2):
                    o0 = hh * SH
                    nc.sync.dma_start(
                        out=rstd_b[:, o0:o0 + SH],
                        in_=bass.AP(tensor=_rt.tensor,
                                    offset=_rt.offset + o0,
                                    ap=[[0, P], [1, SH]]))

            # ------- phase 1: Q/K+RoPE and V, ordered qk0,qk2,V,qk1,qk3 ----
            # so attention (head pair et=0) starts right after qk(0,2); V
            # tiles stream in just ahead of the first AV consumers.
            with tc.tile_pool(name="p1psum", bufs=3, space="PSUM") as p1psum, \
                 tc.tile_pool(name="p1vt", bufs=3) as p1vt, \
                 tc.tile_pool(name="p1tmp", bufs=2) as p1tmp:
                def emit_qk(e, shs=(0, 1)):
                    for sh in shs:
                        s0 = sh * SH
                        zq = p1psum.tile([P, SH], fp32, tag="zqk")
                        for c in range(2):
                            c0 = c * 512
                            for k in range(NK):
                                nc.tensor.matmul(
                                    zq[:, c0:c0 + 512],
                                    wqk_sb[:, e, k, :],
                                    xT_sb[:, k, s0 + c0:s0 + c0 + 512],
                                    start=(k == 0), stop=False)
                            # mean-centering: zq += wsum_e (x) (-mu)
                            nc.tensor.matmul(
                                zq[:, c0:c0 + 512],
                                wsqk_row[0:1, e * P:(e + 1) * P],
                                mu_sb[0:1, s0 + c0:s0 + c0 + 512],
                                start=False, stop=True)
                        # single PSUM read: q pre-scaled by rstd (RoPE
                        # commutes with per-position scalars); k plain cast.
                        # All downstream RoPE ops are bf16 SBUF (DVE 4x).
                        zqn = p1tmp.tile([P, SH], fp32, tag="zqn")
                        if e < 2:
                            nc.vector.tensor_mul(zqn[:], zq[:],
                                                 rstd_b[:, s0:s0 + SH])
                        else:
                            nc.vector.tensor_copy(out=zqn[:], in_=zq[:])
                        # y = zqn * sin2 (sign pre-arranged so the rotate-
                        # half swap happens AFTER the multiply)
                        y = p1tmp.tile([P, SH], fp32, tag="y")
                        nc.vector.tensor_mul(y[:], zqn[:],
                                             sin_sb[:, s0:s0 + SH])
                        # rotate-half swap on the HWDGE rings (gpsimd's
                        # SWDGE ring blocks the Pool engine)
                        ysw = p1tmp.tile([P, SH], fp32, tag="ysw")
                        for g in range(2):
                            b0 = g * HDIM
                            eng = (nc.sync, nc.scalar)[g]
                            eng.dma_start(out=ysw[b0:b0 + HALF, :],
                                          in_=y[b0 + HALF:b0 + HDIM, :])
                            eng.dma_start(out=ysw[b0 + HALF:b0 + HDIM, :],
                                          in_=y[b0:b0 + HALF, :])
                        t1 = p1tmp.tile([P, SH], fp32, tag="t1")
                        nc.vector.tensor_mul(t1[:], zqn[:],
                                             cos_sb[:, s0:s0 + SH])
                        nc.vector.tensor_add(qk_sb[:, e, s0:s0 + SH],
                                             t1[:], ysw[:])

                # qk0's second s-half is deferred into the A(1,0) fill
                # (first needed by A(0,1)); everything else up front.
                # k first: its RoPE needs no rstd, so the rstd bounce
                # round-trips hide behind qk2's matmuls + rotation.
                emit_qk(2)
                emit_qk(0, shs=(0,))

            # -------- phases 2+3: software-pipelined attention ------------
            # emit order == per-engine execute order. Single-HEAD attention
            # blocks use 6 PSUM banks (pscore ring 4 + po 2), leaving 2 for
            # an interleaved qk(1)/qk(3) zq ring (first half) then the
            # out_proj ring (second half). qk(1)/qk(3) and out_proj(i-half
            # 0) are chopped into filler steps drip-fed into the attention
            # j-loops so the PE keeps working while ScalarE exps.
            if _abl == "p01":
                continue
            with tc.tile_pool(name="p1tmp2", bufs=1) as p1tmp2, \
                 tc.tile_pool(name="p2tmp", bufs=4) as p2tmp, \
                 tc.tile_pool(name="p3rec", bufs=1) as p3rec, \
                 tc.tile_pool(name="p3tmp", bufs=3) as p3tmp, \
                 tc.tile_pool(name="p3recd", bufs=2, space="DRAM") as p3recd, \
                 tc.tile_pool(name="ps_s", bufs=2, space="PSUM") as ps_s, \
                 tc.tile_pool(name="ps_o", bufs=1, space="PSUM") as ps_o:

                def qk_sh_steps(e, sh, pool):
                    s0 = sh * SH
                    st = {}

                    def mmchunk(c, k0, k1, first=False):
                        def run():
                            if first:
                                st["zq"] = pool.tile([P, SH], fp32,
                                                     tag="zqk",
                                                     name=f"zq{e}_{sh}")
                            zq = st["zq"]
                            c0 = c * 512
                            for k in range(k0, min(k1, NK)):
                                nc.tensor.matmul(
                                    zq[:, c0:c0 + 512], wqk_sb[:, e, k, :],
                                    xT_sb[:, k, s0 + c0:s0 + c0 + 512],
                                    start=(k == 0), stop=False)
                            if k1 > NK:   # mean-centering caps the group
                                nc.tensor.matmul(
                                    zq[:, c0:c0 + 512],
                                    wsqk_row[0:1, e * P:(e + 1) * P],
                                    mu_sb[0:1, s0 + c0:s0 + c0 + 512],
                                    start=False, stop=True)
                        return run

                    def rope_a():
                        zq = st["zq"]
                        zqn = p1tmp2.tile([P, SH], fp32, tag="zqn")
                        st["zqn"] = zqn
                        if e < 2:
                            nc.vector.tensor_mul(zqn[:], zq[:],
                                                 rstd_b[:, s0:s0 + SH])
                        else:
                            nc.vector.tensor_copy(out=zqn[:], in_=zq[:])
                        y = p1tmp2.tile([P, SH], fp32, tag="y")
                        st["y"] = y
                        nc.vector.tensor_mul(y[:], zqn[:],
                                             sin_sb[:, s0:s0 + SH])

                    def rope_b():
                        zqn, y = st["zqn"], st["y"]
                        ysw = p1tmp2.tile([P, SH], fp32, tag="ysw")
                        for g in range(2):
                            b0 = g * HDIM
                            eng = (nc.sync, nc.scalar)[g]
                            eng.dma_start(out=ysw[b0:b0 + HALF, :],
                                          in_=y[b0 + HALF:b0 + HDIM, :])
                            eng.dma_start(out=ysw[b0 + HALF:b0 + HDIM, :],
                                          in_=y[b0:b0 + HALF, :])
                        t1 = p1tmp2.tile([P, SH], fp32, tag="t1")
                        nc.vector.tensor_mul(t1[:], zqn[:],
                                             cos_sb[:, s0:s0 + SH])
                        nc.vector.tensor_add(qk_sb[:, e, s0:s0 + SH],
                                             t1[:], ysw[:])

                    return [mmchunk(0, 0, 5, first=True), mmchunk(0, 5, 9),
                            mmchunk(1, 0, 5), mmchunk(1, 5, 9),
                            rope_a, rope_b]

                def qk_steps(e, pool):
                    return qk_sh_steps(e, 0, pool) + qk_sh_steps(e, 1, pool)

                def v_steps(pool):
                    # one step per s-tile: zv matmuls + rstd scale into v_sb
                    steps = []
                    for t in range(NS):
                        def run(t=t):
                            zv = pool.tile([P, EC], fp32, tag="zv",
                                           name=f"zv{t}")
                            for k in range(NK):
                                nc.tensor.matmul(
                                    zv[:], xT_sb[:, k, t * P:(t + 1) * P],
                                    wv_sb[:, k, :],
                                    start=(k == 0), stop=(k == NK - 1))
                            # t2v = wvsum * (-mu)_s * rstd_s
                            t2v = p3tmp.tile([P, EC], fp32, tag="t2v",
                                             name=f"t2v{t}")
                            nc.vector.tensor_scalar(out=t2v[:], in0=wsv_b[:],
                                                    scalar1=muTf[:, t:t + 1],
                                                    scalar2=rstdT[:, t:t + 1],
                                                    op0=OP.mult, op1=OP.mult)
                            # v = rstd_s * Zv + t2v
                            nc.vector.scalar_tensor_tensor(
                                out=v_sb[:, t, :].rearrange(
                                    "p (h w) -> p h w", h=HPC)[:, :, 0:HDIM],
                                in0=zv[:].rearrange("p (h d) -> p h d",
                                                    h=HPC),
                                scalar=rstdT[:, t:t + 1],
                                in1=t2v[:].rearrange("p (h d) -> p h d",
                                                     h=HPC),
                                op0=OP.mult, op1=OP.add)
                        steps.append(run)
                    return steps

                def att_block(h, ih, fill=None, pre=0, span=NS - 1):
                    et, hp = divmod(h, 2)
                    ep = hp * HDIM
                    i0 = ih * SH
                    po = ps_o.tile([VW, SH], fp32, tag="po",
                                   name=f"po{h}_{ih}")
                    nfill = len(fill) if fill else 0
                    done = 0
                    while done < pre:
                        fill[done]()
                        done += 1
                    for j in range(NS):
                        # drip-feed filler work BEFORE this j's consumers
                        # (a V fill step t=j must precede AV(j) on the PE)
                        if nfill:
                            tgt = pre + (min(j, span) * (nfill - pre)) // span
                            while done < tgt:
                                fill[done]()
                                done += 1
                        pscore = ps_s.tile([P, SH], fp32, tag="ps")
                        for c in range(2):
                            c0 = c * 512
                            nc.tensor.matmul(
                                pscore[:, c0:c0 + 512],
                                qk_sb[ep:ep + HDIM, 2 + et,
                                      j * P:(j + 1) * P],
                                qk_sb[ep:ep + HDIM, et,
                                      i0 + c0:i0 + c0 + 512],
                                start=True, stop=True)
                        p_sb = p2tmp.tile([P, SH], bf16, tag="p")
                        nc.scalar.activation(p_sb[:], pscore[:], AF.Exp,
                                             scale=rstdT8[:, j:j + 1])
                        for c in range(2):
                            nc.tensor.matmul(
                                po[:, c * 512:(c + 1) * 512],
                                v_sb[:, j, h * VW:(h + 1) * VW],
                                p_sb[:, c * 512:(c + 1) * 512],
                                start=(j == 0), stop=(j == NS - 1),
                                skip_group_check=True)
                    # drain po raw + denominator row (PSUM reads: DVE only;
                    # gpsimd cannot access PSUM). Normalization is deferred
                    # so the po bank recycles immediately.
                    nc.vector.tensor_copy(
                        out=oT_sb[ep:ep + HDIM, et, i0:i0 + SH],
                        in_=po[0:HDIM, :])
                    nc.vector.tensor_copy(out=den_sb[:, ih, et, hp, :],
                                          in_=po[HDIM:HDIM + 1, :])

                def norm(ih, et):
                    i0 = ih * SH
                    rec = p3rec.tile([1, 2, SH], fp32, tag="rec",
                                     name=f"rec{ih}{et}")
                    nc.vector.reciprocal(out=rec[:],
                                         in_=den_sb[:, ih, et, :, :])
                    rec_d = p3recd.tile([1, 2, SH], fp32, tag="recd",
                                        name=f"rec_d{ih}{et}")
                    nc.sync.dma_start(out=rec_d[:], in_=rec[:])
                    recb = p3rec.tile([P, SH], fp32, tag="recb",
                                      name=f"recb{ih}{et}")
                    _rc = rec_d[:]
                    for hp in range(2):
                        nc.sync.dma_start(
                            out=recb[hp * HDIM:(hp + 1) * HDIM, :],
                            in_=bass.AP(tensor=_rc.tensor,
                                        offset=_rc.offset + hp * SH,
                                        ap=[[0, HDIM], [1, SH]]))
                    # normalize oT in place
                    nc.vector.tensor_mul(oT_sb[:, et, i0:i0 + SH],
                                         oT_sb[:, et, i0:i0 + SH],
                                         recb[:])

                def norm_hp(ih, et, hp):
                    # per-head-half normalization: lets the hp=0 chain run
                    # during the last attention block instead of on the
                    # out_proj tail. Reuses the rec/recb tile shapes.
                    i0 = ih * SH
                    ep = hp * HDIM
                    rec = p3rec.tile([1, 2, SH], fp32, tag="rec",
                                     name=f"rech{ih}{et}{hp}")
                    nc.vector.reciprocal(out=rec[:, 0, :],
                                         in_=den_sb[:, ih, et, hp, :])
                    rec_d = p3recd.tile([1, 2, SH], fp32, tag="recd",
                                        name=f"rech_d{ih}{et}{hp}")
                    nc.sync.dma_start(out=rec_d[:, 0, :], in_=rec[:, 0, :])
                    recb = p3rec.tile([P, SH], fp32, tag="recb",
                                      name=f"recbh{ih}{et}{hp}")
                    _rc = rec_d[:]
                    nc.sync.dma_start(
                        out=recb[ep:ep + HDIM, :],
                        in_=bass.AP(tensor=_rc.tensor, offset=_rc.offset,
                                    ap=[[0, HDIM], [1, SH]]))
                    nc.vector.tensor_mul(oT_sb[ep:ep + HDIM, et, i0:i0 + SH],
                                         oT_sb[ep:ep + HDIM, et, i0:i0 + SH],
                                         recb[ep:ep + HDIM, :])

                def outproj_steps(ih, pool):
                    steps = []
                    for th in range(NS // 2):
                        t = ih * (NS // 2) + th

                        def run(t=t):
                            pout = pool.tile([P, D], fp32, tag="pout",
                                             name=f"pout{t}")
                            for c in range(2):
                                for k in range(2):
                                    nc.tensor.matmul(
                                        pout[:, c * 512:(c + 1) * 512],
                                        oT_sb[:, k, t * P:(t + 1) * P],
                                        wo_sb[:, k, c * 512:(c + 1) * 512],
                                        start=(k == 0), stop=(k == 1))
                            ot = p3tmp.tile([P, D], bf16, tag="ot")
                            nc.vector.tensor_copy(out=ot[:], in_=pout[:])
                            eng = nc.sync if t % 2 == 0 else nc.scalar
                            eng.dma_start(out=out[t * P:(t + 1) * P, :],
                                          in_=ot[:])
                        steps.append(run)
                    return steps

                with tc.tile_pool(name="zvp", bufs=2, space="PSUM") as zvp:
                    # V streams as fill: v(t) lands just before AV(j=t)
                    att_block(0, 0, fill=v_steps(zvp), pre=2)
                with tc.tile_pool(name="zqp", bufs=1, space="PSUM") as zqp:
                    # each qk sh-chunk (6 steps ~5us PE) goes to the LATEST
                    # block meeting its consumer deadline, one chunk per
                    # block, to even out per-block PE load. qk3-sh1 (k1
                    # j-tiles 8-15) lands inside A(2,0) with a j<=7
                    # deadline: scores(j>=8) need it.
                    att_block(1, 0, fill=qk_sh_steps(0, 1, zqp), span=13)
                    norm(0, 0)
                    att_block(0, 1, fill=qk_sh_steps(1, 0, zqp), span=13)
                    att_block(1, 1, fill=qk_sh_steps(3, 0, zqp), span=13)
                    norm(1, 0)
                    att_block(2, 0, fill=qk_sh_steps(3, 1, zqp), span=7)
                    att_block(3, 0, fill=qk_sh_steps(1, 1, zqp), span=13)
                    norm(0, 1)
                with tc.tile_pool(name="p3psA", bufs=1,
                                  space="PSUM") as p3psA:
                    op0 = outproj_steps(0, p3psA)
                    att_block(2, 1, fill=op0[:4])
                    norm_hp(1, 1, 0)
                    att_block(3, 1, fill=op0[4:])
                    norm_hp(1, 1, 1)
            with tc.tile_pool(name="p3psB", bufs=3, space="PSUM") as p3psB, \
                 tc.tile_pool(name="p3tmpB", bufs=3) as p3tmpB:
                for th in range(NS // 2):
                    t = NS // 2 + th
                    pout = p3psB.tile([P, D], fp32, tag="pout",
                                      name=f"poutB{t}")
                    for c in range(2):
                        for k in range(2):
                            nc.tensor.matmul(
                                pout[:, c * 512:(c + 1) * 512],
                                oT_sb[:, k, t * P:(t + 1) * P],
                                wo_sb[:, k, c * 512:(c + 1) * 512],
                                start=(k == 0), stop=(k == 1))
                    ot = p3tmpB.tile([P, D], bf16, tag="ot",
                                     name=f"otB{t}")
                    # tail: ScalarE is idle once the last exp retires
                    if t % 2 == 0:
                        nc.vector.tensor_copy(out=ot[:], in_=pout[:])
                    else:
                        nc.scalar.copy(out=ot[:], in_=pout[:])
                    eng = nc.sync if t % 2 == 0 else nc.scalar
                    eng.dma_start(out=out[t * P:(t + 1) * P, :], in_=ot[:])

    nc.compile()
    return nc


def _host_inputs(x, ln_g, ln_b, w_qkv, w_out):
    wq = w_qkv[0:D] * ln_g[None, :]
    wk = w_qkv[D:2 * D] * ln_g[None, :]
    wv = w_qkv[2 * D:3 * D] * ln_g[None, :]
    if np.abs(w_qkv.astype(np.float32) @ ln_b.astype(np.float32)).max() != 0.0:
        raise NotImplementedError("nonzero ln_b not supported")
    inv = 1.0 / (ROPE_THETA ** (np.arange(0, HALF, dtype=np.float32) / HALF))
    fr = np.arange(S, dtype=np.float32)[:, None] * inv[None, :]
    cos = np.cos(fr).T.astype(np.float32)          # [32, S]
    sin = np.sin(fr).T.astype(np.float32)
    # row layout per 64-group: [lo(32); hi(32)]; cos same both halves.
    cosf = np.tile(cos, (4, 1))                    # [128, S]
    # rot_lo = lo*c - hi*s ; rot_hi = hi*c + lo*s. y = zq*sin2 is computed
    # BEFORE the rotate-half swap, so sin2 rows are [+s (lo rows, feeds the
    # hi output after the swap); -s (hi rows, feeds the lo output)].
    sinsg = np.tile(np.concatenate([sin, -sin], 0), (2, 1))
    ins = []
    for core in range(N_CORES):
        b = core // 4
        h0 = (core % 4) * HPC
        sl = slice(h0 * HDIM, (h0 + HPC) * HDIM)
        wq_c, wk_c, wv_c = wq[sl], wk[sl], wv[sl]
        qk = np.concatenate([wq_c, wk_c], 0)
        ins.append({
            "xT": np.ascontiguousarray(x[b].T.astype(BF16)),
            "wqkT": np.ascontiguousarray(qk.T.astype(BF16)),
            "wvT": np.ascontiguousarray(wv_c.T.astype(BF16)),
            "woT": np.ascontiguousarray(w_out[:, sl].T.astype(BF16)),
            "wsum_qk": qk.sum(1).astype(BF16),
            "wvsum": wv_c.sum(1).astype(np.float32),
            "cosf": cosf, "sinsg": sinsg,
        })
    return ins


def kernel(x, ln_g, ln_b, w_qkv, w_out, b_out):
    from concourse import bass_utils
    x = np.asarray(x, np.float32)
    ln_g = np.asarray(ln_g, np.float32)
    ln_b = np.asarray(ln_b, np.float32)
    w_qkv = np.asarray(w_qkv, np.float32)
    w_out = np.asarray(w_out, np.float32)
    b_out = np.asarray(b_out, np.float32)
    if "nc" not in _cache:
        _cache["nc"] = _build()
    ins = _host_inputs(x, ln_g, ln_b, w_qkv, w_out)
    res = bass_utils.run_bass_kernel_spmd(_cache["nc"], ins,
                                          core_ids=list(range(N_CORES)))
    _cache["last_results"] = res
    out = np.zeros((B, S, D), np.float32)
    for core in range(N_CORES):
        out[core // 4] += np.asarray(res.results[core]["out"],
                                     dtype=np.float32)
    out += b_out[None, None, :]
    return out


# revision 64
# speedup vs baseline: 1.1846x; 1.1846x over previous
"""Fused transformer block (LN -> QKV+RoPE -> attention -> out_proj) on 8
Trainium2 NeuronCores.

Sharding: batch (2-way) x heads (4-way) = 8 cores. Core c handles batch
b = c // 4 and the 4 heads starting at 4*(c%4). Each core produces the
out_proj partial sum over its 256 dh-dims; the host sums 4 partials per
batch and adds b_out.

Device math, per core (all matmuls bf16 in / fp32 PSUM accumulate; the
2e-2 tolerance absorbs it):
- x passed transposed: xT [D, S] bf16 (d on partitions, s free).
- all bulk loads issued up front across the sync/gpsimd/scalar DMA
  rings so no weight DMA queues behind a stats-dependent bounce DMA.
- LN stats via TensorE ones-matmuls with x^2 on ScalarE: both engines
  are idle in the pre-attention ramp, so stats pipeline behind the xT
  DMAs for free. mu kept negated: mean-centering is a K=1 matmul
  accumulation folded into the QKV PSUM group.
- q/k RoPE in [e, s] layout; one PSUM read per (e, sh): q is pre-scaled
  by rstd there (RoPE commutes with per-position scalars), k plain-cast;
  k's rstd is folded into exp's per-partition scale. The sin table's
  sign layout lets the rotate-half swap (partition-sliced SBUF->SBUF
  DMAs on the two HWDGE rings) run AFTER the sin multiply.
- software-pipelined emission (per-engine execution is program-order):
  qk0-sh0 + qk2 first (qk0's second s-half defers into the A(1,0)
  fill), then single-HEAD attention blocks (6 PSUM banks:
  pscore ring 2x2 + po 2), leaving 2 banks for the fill rings. V
  streams as fill steps inside the first attention block (v(t) lands
  just before AV(j=t)); qk(1)/qk(3) matmul+RoPE chunks and out_proj
  (i-half 0) tiles drip-feed into later j-loops so the PE works while
  ScalarE exps. ScalarE runs ONLY exp in steady state.
- scores^T[j,i] per (head, j-tile) with K=64; exp on ScalarE
  (scale = rstd_k[j]/8); o^T accumulated over j with lhsT = [v | 1]
  (M=65, row 64 = softmax denominators for free). po + denominator row
  are copied RAW to SBUF (DVE) right after stop; reciprocal (on the
  packed [1,2,SH] rows), DRAM-bounce broadcast, and in-place oT
  normalize run at the earliest (ih, et) completion, off the po-bank
  critical path; the very last normalization splits per head-half so
  half its DRAM round-trip hides inside the final attention block.
- out_proj: i-half 0 interleaved into the last attention blocks; i-half
  1 as a tail on a 3-deep PSUM ring with PSUM->SBUF copies alternating
  DVE/ScalarE (ScalarE is idle after the last exp); host reduces + adds
  b_out.

KREPS env (measurement only): repeats the whole body N times in one
NEFF so device time = d(marginal)/d(KREPS), cancelling the unstable
axon dispatch term. Default 1.
"""
import sys
sys.path.insert(0, "/opt/trn_rl_repo")
import numpy as np
import ml_dtypes
BF16 = ml_dtypes.bfloat16

B, S, D = 2, 2048, 1024
HEADS, HDIM = 16, 64
HALF = HDIM // 2
ROPE_THETA = 10000.0
N_CORES = 8
HPC = HEADS // 4            # heads per core = 4
EC = HPC * HDIM             # per-core q (or k, or v) width = 256
P = 128
NK = D // P                 # 8 d-tiles
NS = S // P                 # 16 s-tiles
VW = HDIM + 1               # v block width incl. ones column = 65
SH = S // 2                 # i-half width = 1024

_cache = {}


def _build():
    import os
    import contextlib
    import concourse.bass as bass
    import concourse.bacc as bacc
    import concourse.tile as tile
    from concourse import mybir
    fp32 = mybir.dt.float32
    bf16 = mybir.dt.bfloat16
    OP = mybir.AluOpType
    AF = mybir.ActivationFunctionType
    _abl = os.environ.get("ABLATE", "")

    nc = bacc.Bacc("TRN2", target_bir_lowering=False, debug=False,
                   enable_asserts=True, num_devices=N_CORES)

    xT = nc.dram_tensor("xT", [D, S], bf16, kind="ExternalInput").ap()
    wqkT = nc.dram_tensor("wqkT", [D, 2 * EC], bf16, kind="ExternalInput").ap()
    wvT = nc.dram_tensor("wvT", [D, EC], bf16, kind="ExternalInput").ap()
    woT = nc.dram_tensor("woT", [EC, D], bf16, kind="ExternalInput").ap()
    wsum_qk = nc.dram_tensor("wsum_qk", [2 * EC], bf16,
                             kind="ExternalInput").ap()
    wvsum = nc.dram_tensor("wvsum", [EC], fp32, kind="ExternalInput").ap()
    cosf = nc.dram_tensor("cosf", [P, S], fp32, kind="ExternalInput").ap()
    sinsg = nc.dram_tensor("sinsg", [P, S], fp32, kind="ExternalInput").ap()
    out = nc.dram_tensor("out", [S, D], bf16, kind="ExternalOutput").ap()

    wqk_r = wqkT.rearrange("(k p) e -> p k e", p=P)
    wv_r = wvT.rearrange("(k p) e -> p k e", p=P)
    wo_r = woT.rearrange("(k p) e -> p k e", p=P)

    reps = int(os.environ.get("KREPS", "1"))
    with tile.TileContext(nc) as tc:
     for _rep in range(reps):
      with tc.tile_pool(name=f"singles{_rep}", bufs=1) as singles, \
           tc.tile_pool(name=f"dram_scr{_rep}", bufs=1,
                        space="DRAM") as dram_scr:
        qk_sb = singles.tile([P, 4, S], bf16)             # 16KB/part
        v_sb = singles.tile([P, NS, HPC * VW], bf16)      # 8.1KB/part
        oT_sb = singles.tile([P, 2, S], bf16)             # raw (unnormalized)
        wo_sb = singles.tile([P, 2, D], bf16)
        den_sb = singles.tile([1, 2, 2, 2, SH], fp32)     # [ih, et, hp, i]
        rstdT = singles.tile([P, NS], fp32)
        muT = singles.tile([P, NS], bf16)
        muTf = singles.tile([P, NS], fp32)
        rstdT8 = singles.tile([P, NS], fp32)
        onep = singles.tile([P, 2], fp32)
        nc.vector.memset(onep[:], 1.0)
        nc.vector.memset(onep[0:1, 1:2], 1e-5)
        eps_b = singles.tile([P, 1], fp32)
        nc.vector.memset(eps_b[:], 1e-5)
        ones_rt = singles.tile([P, 1], bf16)
        nc.vector.tensor_copy(out=ones_rt[:], in_=onep[:, 0:1])
        ones_sb = ones_rt[:]
        eps_sb = onep[0:1, 1:2]
        # ones column of [v | 1]
        nc.vector.tensor_copy(
            out=v_sb[:].rearrange("p t (h w) -> p t h w", w=VW)[:, :, :,
                                                               HDIM:VW],
            in_=onep[:, 0:1].broadcast_to([P, NS, HPC, 1]))

        with tc.tile_pool(name="ph1a", bufs=1) as ph1a:
            # ---------------- bulk loads, consumption order ----------------
            xT_sb = ph1a.tile([P, NK, S], bf16)           # 32KB/part
            xT_r = xT.rearrange("(k p) s -> p k s", p=P)
            for k in range(NK):     # split across the two HWDGE rings
                eng = (nc.sync, nc.scalar)[k % 2]
                eng.dma_start(out=xT_sb[:, k, :], in_=xT_r[:, k, :])
            wv_sb = ph1a.tile([P, NK, EC], bf16)          # 4KB/part
            for k2 in range(2):
                nc.sync.dma_start(out=wv_sb[:, 4 * k2:4 * (k2 + 1), :],
                                  in_=wv_r[:, 4 * k2:4 * (k2 + 1), :])
            wqk_sb = ph1a.tile([P, 4, NK, P], bf16)       # 8KB/part
            for e in range(4):
                nc.scalar.dma_start(out=wqk_sb[:, e, :, :],
                                    in_=wqk_r[:, :, e * P:(e + 1) * P])
            wsqk_row = ph1a.tile([1, 2 * EC], bf16)
            nc.sync.dma_start(out=wsqk_row[:], in_=wsum_qk.unsqueeze(0))
            wsv_b = ph1a.tile([P, EC], fp32)
            nc.sync.dma_start(
                out=wsv_b[:],
                in_=bass.AP(tensor=wvsum.tensor, offset=wvsum.offset,
                            ap=[[0, P], [1, EC]]))
            cos_sb = ph1a.tile([P, S], fp32)
            sin_sb = ph1a.tile([P, S], fp32)
            nc.scalar.dma_start(out=cos_sb[:], in_=cosf)
            nc.sync.dma_start(out=sin_sb[:], in_=sinsg)
            for k in range(2):
                nc.sync.dma_start(out=wo_sb[:, k, :], in_=wo_r[:, k, :])
            rstd_b = ph1a.tile([P, S], fp32)
            mu_sb = ph1a.tile([1, S], bf16)           # holds -mu

            # ---------------- phase 0: LN stats ----------------
            with tc.tile_pool(name="p0ps_a", bufs=2, space="PSUM") as p0ps_a, \
                 tc.tile_pool(name="p0ps_b", bufs=1, space="PSUM") as p0ps_b, \
                 tc.tile_pool(name="p0scr", bufs=1) as p0scr, \
                 tc.tile_pool(name="p0tmp", bufs=2) as p0tmp:
                ssq_sb = p0scr.tile([1, S], fp32)
                rstd_sb = p0scr.tile([1, S], fp32)
                # stats PE matmuls + ScalarE Square both live in T0, where
                # both engines are otherwise idle (first exp is ~50us away);
                # they pipeline behind the xT DMAs for free.
                for c in range(4):
                    ps_sum = p0ps_a.tile([1, 512], fp32, tag="ps")
                    for k in range(NK):
                        nc.tensor.matmul(ps_sum[:], ones_sb,
                                         xT_sb[:, k, c * 512:(c + 1) * 512],
                                         start=(k == 0), stop=(k == NK - 1))
                    nc.vector.tensor_scalar_mul(mu_sb[:, c * 512:(c + 1) * 512],
                                                ps_sum[:], -1.0 / D)
                # bounce mu + gather its transpose NOW: the DMA round
                # trip overlaps the whole x^2/psq phase below.
                mu_d = dram_scr.tile([1, S], bf16)
                nc.sync.dma_start(out=mu_d[:], in_=mu_sb[:])
                _md = mu_d[:]
                nc.sync.dma_start(
                    out=muT[:],
                    in_=bass.AP(tensor=_md.tensor, offset=_md.offset,
                                ap=[[1, P], [P, NS]]))
                nc.vector.tensor_copy(out=muTf[:], in_=muT[:])
                psq = [p0ps_b.tile([1, 512], fp32, tag=f"psq{c}", name=f"psq{c}")
                       for c in range(4)]
                for k in range(NK):
                    for h2 in range(2):
                        xsq = p0tmp.tile([P, S // 2], bf16, tag="xsq")
                        nc.scalar.activation(
                            xsq[:], xT_sb[:, k, h2 * 1024:(h2 + 1) * 1024],
                            AF.Square)
                        for c in range(2):
                            ci = h2 * 2 + c
                            nc.tensor.matmul(psq[ci][:], ones_sb,
                                             xsq[:, c * 512:(c + 1) * 512],
                                             start=(k == 0), stop=(k == NK - 1),
                                             skip_group_check=True)
                for c in range(4):
                    nc.vector.tensor_copy(out=ssq_sb[:, c * 512:(c + 1) * 512],
                                          in_=psq[c][:])
                # rstd computed in the TRANSPOSED [128, NS] layout: the
                # row-form chain costs ~2us per [1, S] op (partition-
                # starved); here every op is free-size NS=16 (~70ns).
                # Bounce mu/ssq rows -> DRAM, gather transposed, compute
                # rstdT, bounce back, and gather the rstd_b broadcast
                # straight from the transposed DRAM image.
                ssq_d = dram_scr.tile([1, S], fp32)
                nc.sync.dma_start(out=ssq_d[:], in_=ssq_sb[:])
                _sd = ssq_d[:]
                ssqT = p0scr.tile([P, NS], fp32)
                nc.sync.dma_start(
                    out=ssqT[:],
                    in_=bass.AP(tensor=_sd.tensor, offset=_sd.offset,
                                ap=[[1, P], [P, NS]]))
                # var = ssq/D - mu^2 ; rstd = 1/sqrt(var + eps)
                nc.vector.tensor_mul(rstdT[:], muTf[:], muTf[:])
                nc.vector.scalar_tensor_tensor(out=rstdT[:], in0=ssqT[:],
                                               scalar=1.0 / D, in1=rstdT[:],
                                               op0=OP.mult, op1=OP.subtract)
                nc.scalar.activation(rstdT[:], rstdT[:], AF.Sqrt,
                                     bias=eps_b[:])
                nc.vector.reciprocal(out=rstdT[:], in_=rstdT[:])
                nc.vector.tensor_scalar_mul(rstdT8[:], rstdT[:],
                                            float(HDIM) ** -0.5)
                # broadcast form: bounce rstdT's [p, t] image and gather
                # rows with s = t*128 + p -> addr p*NS + t (3D AP).
                rstd_td = dram_scr.tile([1, S], fp32)
                _rt = rstd_td[:]
                # transposed WRITE: rstdT[p, t] lands at s = t*128 + p, so
                # DRAM holds rstd in natural s-order for the broadcasts.
                nc.sync.dma_start(
                    out=bass.AP(tensor=_rt.tensor, offset=_rt.offset,
                                ap=[[1, P], [P, NS]]),
                    in_=rstdT[:])
                for hh in range(2):
                    o0 = hh * SH
                    nc.sync.dma_start(
                        out=rstd_b[:, o0:o0 + SH],
                        in_=bass.AP(tensor=_rt.tensor,
                                    offset=_rt.offset + o0,
                                    ap=[[0, P], [1, SH]]))

            # ------- phase 1: Q/K+RoPE and V, ordered qk0,qk2,V,qk1,qk3 ----
            # so attention (head pair et=0) starts right after qk(0,2); V
            # tiles stream in just ahead of the first AV consumers.
            with tc.tile_pool(name="p1psum", bufs=3, space="PSUM") as p1psum, \
                 tc.tile_pool(name="p1vt", bufs=3) as p1vt, \
                 tc.tile_pool(name="p1tmp", bufs=2) as p1tmp:
                def emit_qk(e, shs=(0, 1)):
                    for sh in shs:
                        s0 = sh * SH
                        zq = p1psum.tile([P, SH], fp32, tag="zqk")
                        for c in range(2):
                            c0 = c * 512
                            for k in range(NK):
                                nc.tensor.matmul(
                                    zq[:, c0:c0 + 512],
                                    wqk_sb[:, e, k, :],
                                    xT_sb[:, k, s0 + c0:s0 + c0 + 512],
                                    start=(k == 0), stop=False)
                            # mean-centering: zq += wsum_e (x) (-mu)
                            nc.tensor.matmul(
                                zq[:, c0:c0 + 512],
                                wsqk_row[0:1, e * P:(e + 1) * P],
                                mu_sb[0:1, s0 + c0:s0 + c0 + 512],
                                start=False, stop=True)
                        # single PSUM read: q pre-scaled by rstd (RoPE
                        # commutes with per-position scalars); k plain cast.
                        # All downstream RoPE ops are bf16 SBUF (DVE 4x).
                        zqn = p1tmp.tile([P, SH], fp32, tag="zqn")
                        if e < 2:
                            nc.vector.tensor_mul(zqn[:], zq[:],
                                                 rstd_b[:, s0:s0 + SH])
                        else:
                            nc.vector.tensor_copy(out=zqn[:], in_=zq[:])
                        # y = zqn * sin2 (sign pre-arranged so the rotate-
                        # half swap happens AFTER the multiply)
                        y = p1tmp.tile([P, SH], fp32, tag="y")
                        nc.vector.tensor_mul(y[:], zqn[:],
                                             sin_sb[:, s0:s0 + SH])
                        # rotate-half swap on the HWDGE rings (gpsimd's
                        # SWDGE ring blocks the Pool engine)
                        ysw = p1tmp.tile([P, SH], fp32, tag="ysw")
                        for g in range(2):
                            b0 = g * HDIM
                            eng = (nc.sync, nc.scalar)[g]
                            eng.dma_start(out=ysw[b0:b0 + HALF, :],
                                          in_=y[b0 + HALF:b0 + HDIM, :])
                            eng.dma_start(out=ysw[b0 + HALF:b0 + HDIM, :],
                                          in_=y[b0:b0 + HALF, :])
                        t1 = p1tmp.tile([P, SH], fp32, tag="t1")
                        nc.vector.tensor_mul(t1[:], zqn[:],
                                             cos_sb[:, s0:s0 + SH])
                        nc.vector.tensor_add(qk_sb[:, e, s0:s0 + SH],
                                             t1[:], ysw[:])

                # qk0's second s-half is deferred into the A(1,0) fill
                # (first needed by A(0,1)); everything else up front.
                # k first: its RoPE needs no rstd, so the rstd bounce
                # round-trips hide behind qk2's matmuls + rotation.
                emit_qk(2)
                emit_qk(0, shs=(0,))

            # -------- phases 2+3: software-pipelined attention ------------
            # emit order == per-engine execute order. Single-HEAD attention
            # blocks use 6 PSUM banks (pscore ring 4 + po 2), leaving 2 for
            # an interleaved qk(1)/qk(3) zq ring (first half) then the
            # out_proj ring (second half). qk(1)/qk(3) and out_proj(i-half
            # 0) are chopped into filler steps drip-fed into the attention
            # j-loops so the PE keeps working while ScalarE exps.
            if _abl == "p01":
                continue
            with tc.tile_pool(name="p1tmp2", bufs=1) as p1tmp2, \
                 tc.tile_pool(name="p2tmp", bufs=4) as p2tmp, \
                 tc.tile_pool(name="p3rec", bufs=1) as p3rec, \
                 tc.tile_pool(name="p3tmp", bufs=3) as p3tmp, \
                 tc.tile_pool(name="p3recd", bufs=2, space="DRAM") as p3recd, \
                 tc.tile_pool(name="ps_s", bufs=2, space="PSUM") as ps_s, \
                 tc.tile_pool(name="ps_o", bufs=1, space="PSUM") as ps_o:

                def qk_sh_steps(e, sh, pool):
                    s0 = sh * SH
                    st = {}

                    def mmchunk(c, k0, k1, first=False):
                        def run():
                            if first:
                                st["zq"] = pool.tile([P, SH], fp32,
                                                     tag="zqk",
                                                     name=f"zq{e}_{sh}")
                            zq = st["zq"]
                            c0 = c * 512
                            for k in range(k0, min(k1, NK)):
                                nc.tensor.matmul(
                                    zq[:, c0:c0 + 512], wqk_sb[:, e, k, :],
                                    xT_sb[:, k, s0 + c0:s0 + c0 + 512],
                                    start=(k == 0), stop=False)
                            if k1 > NK:   # mean-centering caps the group
                                nc.tensor.matmul(
                                    zq[:, c0:c0 + 512],
                                    wsqk_row[0:1, e * P:(e + 1) * P],
                                    mu_sb[0:1, s0 + c0:s0 + c0 + 512],
                                    start=False, stop=True)
                        return run

                    def rope_a():
                        zq = st["zq"]
                        zqn = p1tmp2.tile([P, SH], fp32, tag="zqn")
                        st["zqn"] = zqn
                        if e < 2:
                            nc.vector.tensor_mul(zqn[:], zq[:],
                                                 rstd_b[:, s0:s0 + SH])
                        else:
                            nc.vector.tensor_copy(out=zqn[:], in_=zq[:])
                        y = p1tmp2.tile([P, SH], fp32, tag="y")
                        st["y"] = y
                        nc.vector.tensor_mul(y[:], zqn[:],
                                             sin_sb[:, s0:s0 + SH])

                    def rope_b():
                        zqn, y = st["zqn"], st["y"]
                        ysw = p1tmp2.tile([P, SH], fp32, tag="ysw")
                        for g in range(2):
                            b0 = g * HDIM
                            eng = (nc.sync, nc.scalar)[g]
                            eng.dma_start(out=ysw[b0:b0 + HALF, :],
                                          in_=y[b0 + HALF:b0 + HDIM, :])
                            eng.dma_start(out=ysw[b0 + HALF:b0 + HDIM, :],
                                          in_=y[b0:b0 + HALF, :])
                        t1 = p1tmp2.tile([P, SH], fp32, tag="t1")
                        nc.vector.tensor_mul(t1[:], zqn[:],
                                             cos_sb[:, s0:s0 + SH])
                        nc.vector.tensor_add(qk_sb[:, e, s0:s0 + SH],
                                             t1[:], ysw[:])

                    return [mmchunk(0, 0, 5, first=True), mmchunk(0, 5, 9),
                            mmchunk(1, 0, 5), mmchunk(1, 5, 9),
                            rope_a, rope_b]

                def qk_steps(e, pool):
                    return qk_sh_steps(e, 0, pool) + qk_sh_steps(e, 1, pool)

                def v_steps(pool):
                    # one step per s-tile: zv matmuls + rstd scale into v_sb
                    steps = []
                    for t in range(NS):
                        def run(t=t):
                            zv = pool.tile([P, EC], fp32, tag="zv",
                                           name=f"zv{t}")
                            for k in range(NK):
                                nc.tensor.matmul(
                                    zv[:], xT_sb[:, k, t * P:(t + 1) * P],
                                    wv_sb[:, k, :],
                                    start=(k == 0), stop=(k == NK - 1))
                            # t2v = wvsum * (-mu)_s * rstd_s
                            t2v = p3tmp.tile([P, EC], fp32, tag="t2v",
                                             name=f"t2v{t}")
                            nc.vector.tensor_scalar(out=t2v[:], in0=wsv_b[:],
                                                    scalar1=muTf[:, t:t + 1],
                                                    scalar2=rstdT[:, t:t + 1],
                                                    op0=OP.mult, op1=OP.mult)
                            # v = rstd_s * Zv + t2v
                            nc.vector.scalar_tensor_tensor(
                                out=v_sb[:, t, :].rearrange(
                                    "p (h w) -> p h w", h=HPC)[:, :, 0:HDIM],
                                in0=zv[:].rearrange("p (h d) -> p h d",
                                                    h=HPC),
                                scalar=rstdT[:, t:t + 1],
                                in1=t2v[:].rearrange("p (h d) -> p h d",
                                                     h=HPC),
                                op0=OP.mult, op1=OP.add)
                        steps.append(run)
                    return steps

                def att_block(h, ih, fill=None, pre=0, span=NS - 1):
                    et, hp = divmod(h, 2)
                    ep = hp * HDIM
                    i0 = ih * SH
                    po = ps_o.tile([VW, SH], fp32, tag="po",
                                   name=f"po{h}_{ih}")
                    nfill = len(fill) if fill else 0
                    done = 0
                    while done < pre:
                        fill[done]()
                        done += 1
                    for j in range(NS):
                        # drip-feed filler work BEFORE this j's consumers
                        # (a V fill step t=j must precede AV(j) on the PE)
                        if nfill:
                            tgt = pre + (min(j, span) * (nfill - pre)) // span
                            while done < tgt:
                                fill[done]()
                                done += 1
                        pscore = ps_s.tile([P, SH], fp32, tag="ps")
                        for c in range(2):
                            c0 = c * 512
                            nc.tensor.matmul(
                                pscore[:, c0:c0 + 512],
                                qk_sb[ep:ep + HDIM, 2 + et,
                                      j * P:(j + 1) * P],
                                qk_sb[ep:ep + HDIM, et,
                                      i0 + c0:i0 + c0 + 512],
                                start=True, stop=True)
                        p_sb = p2tmp.tile([P, SH], bf16, tag="p")
                        nc.scalar.activation(p_sb[:], pscore[:], AF.Exp,
                                             scale=rstdT8[:, j:j + 1])
                        for c in range(2):
                            nc.tensor.matmul(
                                po[:, c * 512:(c + 1) * 512],
                                v_sb[:, j, h * VW:(h + 1) * VW],
                                p_sb[:, c * 512:(c + 1) * 512],
                                start=(j == 0), stop=(j == NS - 1),
                                skip_group_check=True)
                    # drain po raw + denominator row (PSUM reads: DVE only;
                    # gpsimd cannot access PSUM). Normalization is deferred
                    # so the po bank recycles immediately.
                    nc.vector.tensor_copy(
                        out=oT_sb[ep:ep + HDIM, et, i0:i0 + SH],
                        in_=po[0:HDIM, :])
                    nc.vector.tensor_copy(out=den_sb[:, ih, et, hp, :],
                                          in_=po[HDIM:HDIM + 1, :])

                def norm(ih, et):
                    i0 = ih * SH
                    rec = p3rec.tile([1, 2, SH], fp32, tag="rec",
                                     name=f"rec{ih}{et}")
                    nc.vector.reciprocal(out=rec[:],
                                         in_=den_sb[:, ih, et, :, :])
                    rec_d = p3recd.tile([1, 2, SH], fp32, tag="recd",
                                        name=f"rec_d{ih}{et}")
                    nc.sync.dma_start(out=rec_d[:], in_=rec[:])
                    recb = p3rec.tile([P, SH], fp32, tag="recb",
                                      name=f"recb{ih}{et}")
                    _rc = rec_d[:]
                    for hp in range(2):
                        nc.sync.dma_start(
                            out=recb[hp * HDIM:(hp + 1) * HDIM, :],
                            in_=bass.AP(tensor=_rc.tensor,
                                        offset=_rc.offset + hp * SH,
                                        ap=[[0, HDIM], [1, SH]]))
                    # normalize oT in place
                    nc.vector.tensor_mul(oT_sb[:, et, i0:i0 + SH],
                                         oT_sb[:, et, i0:i0 + SH],
                                         recb[:])

                def norm_hp(ih, et, hp):
                    # per-head-half normalization: lets the hp=0 chain run
                    # during the last attention block instead of on the
                    # out_proj tail. Reuses the rec/recb tile shapes.
                    i0 = ih * SH
                    ep = hp * HDIM
                    rec = p3rec.tile([1, 2, SH], fp32, tag="rec",
                                     name=f"rech{ih}{et}{hp}")
                    nc.vector.reciprocal(out=rec[:, 0, :],
                                         in_=den_sb[:, ih, et, hp, :])
                    rec_d = p3recd.tile([1, 2, SH], fp32, tag="recd",
                                        name=f"rech_d{ih}{et}{hp}")
                    nc.sync.dma_start(out=rec_d[:, 0, :], in_=rec[:, 0, :])
                    recb = p3rec.tile([P, SH], fp32, tag="recb",
                                      name=f"recbh{ih}{et}{hp}")
                    _rc = rec_d[:]
                    nc.sync.dma_start(
                        out=recb[ep:ep + HDIM, :],
                        in_=bass.AP(tensor=_rc.tensor, offset=_rc.offset,
                                    ap=[[0, HDIM], [1, SH]]))
                    nc.vector.tensor_mul(oT_sb[ep:ep + HDIM, et, i0:i0 + SH],
                                         oT_sb[ep:ep + HDIM, et, i0:i0 + SH],
                                         recb[ep:ep + HDIM, :])

                def outproj_steps(ih, pool):
                    steps = []
                    for th in range(NS // 2):
                        t = ih * (NS // 2) + th

                        def run(t=t):
                            pout = pool.tile([P, D], fp32, tag="pout",
                                             name=f"pout{t}")
                            for c in range(2):
                                for k in range(2):
                                    nc.tensor.matmul(
                                        pout[:, c * 512:(c + 1) * 512],
                                        oT_sb[:, k, t * P:(t + 1) * P],
                                        wo_sb[:, k, c * 512:(c + 1) * 512],
                                        start=(k == 0), stop=(k == 1))
                            ot = p3tmp.tile([P, D], bf16, tag="ot")
                            nc.vector.tensor_copy(out=ot[:], in_=pout[:])
                            eng = nc.sync if t % 2 == 0 else nc.scalar
                            eng.dma_start(out=out[t * P:(t + 1) * P, :],
                                          in_=ot[:])
                        steps.append(run)
                    return steps

                with tc.tile_pool(name="zvp", bufs=2, space="PSUM") as zvp:
                    # V streams as fill: v(t) lands just before AV(j=t)
                    att_block(0, 0, fill=v_steps(zvp), pre=2)
                with tc.tile_pool(name="zqp", bufs=1, space="PSUM") as zqp:
                    # each qk sh-chunk (6 steps ~5us PE) goes to the LATEST
                    # block meeting its consumer deadline, one chunk per
                    # block, to even out per-block PE load. qk3-sh1 (k1
                    # j-tiles 8-15) lands inside A(2,0) with a j<=7
                    # deadline: scores(j>=8) need it.
                    att_block(1, 0, fill=qk_sh_steps(0, 1, zqp), span=13)
                    norm(0, 0)
                    att_block(0, 1, fill=qk_sh_steps(1, 0, zqp), span=13)
                    att_block(1, 1, fill=qk_sh_steps(3, 0, zqp), span=13)
                    norm(1, 0)
                    att_block(2, 0, fill=qk_sh_steps(3, 1, zqp), span=7)
                    att_block(3, 0, fill=qk_sh_steps(1, 1, zqp), span=13)
                    norm(0, 1)
                with tc.tile_pool(name="p3psA", bufs=1,
                                  space="PSUM") as p3psA:
                    op0 = outproj_steps(0, p3psA)
                    att_block(2, 1, fill=op0[:4])
                    norm_hp(1, 1, 0)
                    att_block(3, 1, fill=op0[4:])
                    norm_hp(1, 1, 1)
            with tc.tile_pool(name="p3psB", bufs=3, space="PSUM") as p3psB, \
                 tc.tile_pool(name="p3tmpB", bufs=3) as p3tmpB:
                # software-pipelined depth-3: each tile's k=0 matmuls (no
                # dependence on the final normalization) are emitted ahead,
                # so they run during the last norm chain's DRAM round-trip
                # instead of stalling behind the k=1 semaphore.
                pend = []

                def flushB():
                    t, pout = pend.pop(0)
                    for c in range(2):
                        nc.tensor.matmul(
                            pout[:, c * 512:(c + 1) * 512],
                            oT_sb[:, 1, t * P:(t + 1) * P],
                            wo_sb[:, 1, c * 512:(c + 1) * 512],
                            start=False, stop=True, skip_group_check=True)
                    ot = p3tmpB.tile([P, D], bf16, tag="ot",
                                     name=f"otB{t}")
                    # tail: ScalarE is idle once the last exp retires
                    if t % 2 == 0:
                        nc.vector.tensor_copy(out=ot[:], in_=pout[:])
                    else:
                        nc.scalar.copy(out=ot[:], in_=pout[:])
                    eng = nc.sync if t % 2 == 0 else nc.scalar
                    eng.dma_start(out=out[t * P:(t + 1) * P, :], in_=ot[:])

                for th in range(NS // 2):
                    t = NS // 2 + th
                    pout = p3psB.tile([P, D], fp32, tag="pout",
                                      name=f"poutB{t}")
                    for c in range(2):
                        nc.tensor.matmul(
                            pout[:, c * 512:(c + 1) * 512],
                            oT_sb[:, 0, t * P:(t + 1) * P],
                            wo_sb[:, 0, c * 512:(c + 1) * 512],
                            start=True, stop=False, skip_group_check=True)
                    pend.append((t, pout))
                    if len(pend) == 3:
                        flushB()
                while pend:
                    flushB()

    nc.compile()
    return nc


def _host_inputs(x, ln_g, ln_b, w_qkv, w_out):
    wq = w_qkv[0:D] * ln_g[None, :]
    wk = w_qkv[D:2 * D] * ln_g[None, :]
    wv = w_qkv[2 * D:3 * D] * ln_g[None, :]
    if np.abs(w_qkv.astype(np.float32) @ ln_b.astype(np.float32)).max() != 0.0:
        raise NotImplementedError("nonzero ln_b not supported")
    inv = 1.0 / (ROPE_THETA ** (np.arange(0, HALF, dtype=np.float32) / HALF))
    fr = np.arange(S, dtype=np.float32)[:, None] * inv[None, :]
    cos = np.cos(fr).T.astype(np.float32)          # [32, S]
    sin = np.sin(fr).T.astype(np.float32)
    # row layout per 64-group: [lo(32); hi(32)]; cos same both halves.
    cosf = np.tile(cos, (4, 1))                    # [128, S]
    # rot_lo = lo*c - hi*s ; rot_hi = hi*c + lo*s. y = zq*sin2 is computed
    # BEFORE the rotate-half swap, so sin2 rows are [+s (lo rows, feeds the
    # hi output after the swap); -s (hi rows, feeds the lo output)].
    sinsg = np.tile(np.concatenate([sin, -sin], 0), (2, 1))
    ins = []
    for core in range(N_CORES):
        b = core // 4
        h0 = (core % 4) * HPC
        sl = slice(h0 * HDIM, (h0 + HPC) * HDIM)
        wq_c, wk_c, wv_c = wq[sl], wk[sl], wv[sl]
        qk = np.concatenate([wq_c, wk_c], 0)
        ins.append({
            "xT": np.ascontiguousarray(x[b].T.astype(BF16)),
            "wqkT": np.ascontiguousarray(qk.T.astype(BF16)),
            "wvT": np.ascontiguousarray(wv_c.T.astype(BF16)),
            "woT": np.ascontiguousarray(w_out[:, sl].T.astype(BF16)),
            "wsum_qk": qk.sum(1).astype(BF16),
            "wvsum": wv_c.sum(1).astype(np.float32),
            "cosf": cosf, "sinsg": sinsg,
        })
    return ins


def kernel(x, ln_g, ln_b, w_qkv, w_out, b_out):
    from concourse import bass_utils
    x = np.asarray(x, np.float32)
    ln_g = np.asarray(ln_g, np.float32)
    ln_b = np.asarray(ln_b, np.float32)
    w_qkv = np.asarray(w_qkv, np.float32)
    w_out = np.asarray(w_out, np.float32)
    b_out = np.asarray(b_out, np.float32)
    if "nc" not in _cache:
        _cache["nc"] = _build()
    ins = _host_inputs(x, ln_g, ln_b, w_qkv, w_out)
    res = bass_utils.run_bass_kernel_spmd(_cache["nc"], ins,
                                          core_ids=list(range(N_CORES)))
    _cache["last_results"] = res
    out = np.zeros((B, S, D), np.float32)
    for core in range(N_CORES):
        out[core // 4] += np.asarray(res.results[core]["out"],
                                     dtype=np.float32)
    out += b_out[None, None, :]
    return out
